# revision 1
# baseline (speedup 1.0000x reference)
"""ConvCaps2D kernel for 8 Trainium2 NeuronCores.

Sharding (hardcoded, per spec sharding_hint): data-parallel over
(batch=4) x (H-halves=2) = 8 shards. Each core gets 17 input rows
(15 output rows + kh-1=2 halo), the full W dim, and the replicated
small kernel. Routing is purely local per output position, so there
is no cross-core communication.

Inputs (full, unsharded):
  input_poses       f32[4, 32, 32, 16, 4, 4]
  input_activations f32[4, 32, 32, 16]   (unused by the reference math)
  kernel            f32[16, 3, 3, 16, 4, 4]
Returns (full):
  capsule_poses       f32[4, 30, 30, 16, 4, 4]
  capsule_activations f32[4, 30, 30, 16]
"""

import jax
import jax.numpy as jnp
import numpy as np
from functools import partial

R_ITERS = 3
EPS = 1e-9

B, H, W, C, P, Q = 4, 32, 32, 16, 4, 4
N, KH, KW, RR = 16, 3, 3, 4
HOUT = H - KH + 1  # 30
WOUT = W - KW + 1  # 30
HHALF = HOUT // 2  # 15 output rows per shard
NCORES = 8


def _squash(s):
    n2 = jnp.sum(s * s, axis=-1, keepdims=True)
    return s * (n2 / (1.0 + n2)) / jnp.sqrt(n2 + EPS)


def _local_convcaps(poses, kflat):
    """poses: [17, 32, C, P, Q] local shard; kflat: [N, KH*KW, C, Q, R].
    Returns (capsule_poses [15, 30, N, P, R], acts [15, 30, N])."""
    hout = poses.shape[0] - KH + 1
    wout = poses.shape[1] - KW + 1
    blocks = jnp.stack(
        [poses[i:i + hout, j:j + wout] for i in range(KH) for j in range(KW)],
        axis=2,
    )  # [hout, wout, 9, C, P, Q]
    votes = jnp.einsum('hwkcpq,nkcqr->hwnkcpr', blocks, kflat)
    M = KH * KW * C
    vflat = votes.reshape(hout, wout, N, M, P * RR)

    b = jnp.zeros((hout, wout, N, M), dtype=vflat.dtype)
    for _ in range(R_ITERS):
        c = jax.nn.softmax(b, axis=2)
        s = jnp.einsum('hwnm,hwnmd->hwnd', c, vflat)
        v = _squash(s)
        b = b + jnp.einsum('hwnmd,hwnd->hwnm', vflat, v)

    capsule_poses = v.reshape(hout, wout, N, P, RR)
    vec = _squash(v)
    capsule_activations = jnp.sqrt(jnp.sum(vec * vec, axis=-1) + EPS)
    return capsule_poses, capsule_activations


@partial(jax.pmap, axis_name='i')
def _pmapped(poses_shard, kflat):
    return _local_convcaps(poses_shard, kflat)


def kernel(input_poses, input_activations, kernel):
    del input_activations  # not used by the reference computation
    kflat = np.asarray(kernel, np.float32).reshape(N, KH * KW, C, Q, RR)

    # Build 8 shards: (b, h-half) -> input rows [h0, h0+17)
    shards = np.empty((NCORES, HHALF + KH - 1, W, C, P, Q), np.float32)
    ip = np.asarray(input_poses, np.float32)
    for core in range(NCORES):
        b, half = divmod(core, 2)
        h0 = half * HHALF
        shards[core] = ip[b, h0:h0 + HHALF + KH - 1]

    kb = np.broadcast_to(kflat, (NCORES,) + kflat.shape)
    pose_sh, act_sh = _pmapped(jnp.asarray(shards), jnp.asarray(kb))
    pose_sh = np.asarray(pose_sh)
    act_sh = np.asarray(act_sh)

    out_poses = np.empty((B, HOUT, WOUT, N, P, RR), np.float32)
    out_acts = np.empty((B, HOUT, WOUT, N), np.float32)
    for core in range(NCORES):
        b, half = divmod(core, 2)
        h0 = half * HHALF
        out_poses[b, h0:h0 + HHALF] = pose_sh[core]
        out_acts[b, h0:h0 + HHALF] = act_sh[core]
    return out_poses, out_acts
